# revision 38
# baseline (speedup 1.0000x reference)
"""Trainium2 Bass kernel for CausalSelfAttention (T=4096, D=768, H=6, hd=128).

Sharding: query rows are strided across the 8 cores (core c owns rows
c, c+8, c+16, ...), which balances the causal-attention work exactly and
keeps the compiled program identical on every core (SPMD); all per-core
differences are carried by the input data (sliced x columns, rotary rows,
mask tiles). K/V projections are replicated on every core, so there is no
cross-core communication at all.

Layouts on device (per core):
  - scores are computed transposed: scoresT[s, q] so that attn @ V needs no
    on-chip transposes (V arrives naturally as [s, dh]).
  - Q is projected naturally [q, dh], RMS-normed + rotated there (free-dim
    reductions), then PE-transposed once (it is small).
  - K is projected directly transposed KT[dh, s]; its RMS scale commutes
    with rotary and is folded into the softmax exp() per-partition scale.
  - softmax denominators come from a ones-vector matmul; normalization is
    folded into the PSUM->SBUF copy of the attention output.
All matmuls run as float32r (full-rate fp32 on the PE array).
"""
import ml_dtypes
import numpy as np
from contextlib import ExitStack

import concourse.bass as bass
import concourse.tile as tile
from concourse import bacc, bass_isa, mybir
from concourse.bass import ts
from concourse.masks import make_identity

T, D, H, HD = 4096, 768, 6, 128
NCORES = 8
QPC = T // NCORES            # 512 query rows per core
NG, HPG = 3, 2               # head groups x heads per group
EPS = float(np.finfo(np.float32).eps)
F32 = mybir.dt.float32
F32R = mybir.dt.float32r
AF = mybir.ActivationFunctionType
ALU = mybir.AluOpType


def build_program(stage="full"):
    nc = bacc.Bacc("TRN2", target_bir_lowering=False, debug=False,
                   enable_asserts=False, num_devices=NCORES)
    dt_in = {}
    def din(name, shape):
        dt_in[name] = nc.dram_tensor(name, shape, F32, kind="ExternalInput").ap()
        return dt_in[name]

    xT = din("xT", [D, T])
    xTq = din("xTq", [D, QPC])
    wqT = din("wqT", [D, D])
    wkT = din("wkT", [D, D])
    wvT = din("wvT", [D, D])          # pre-scaled by (1-lamb) on host
    woT = din("woT", [D, D])
    vres = din("vres", [T, D])        # pre-scaled by lamb on host
    cossinT = din("cossinT", [64, T])  # rows 0:32 cos, 32:64 sin (K side)
    cossinQ = din("cossinQ", [QPC, 64])  # cols 0:32 cos, 32:64 sin (q rows)
    ones_in = din("ones_in", [128, 1])
    masks = nc.dram_tensor("masks", [16, 128, 256], mybir.dt.bfloat16,
                           kind="ExternalInput").ap()
    out = nc.dram_tensor("out", [QPC, D], F32, kind="ExternalOutput").ap()

    with tile.TileContext(nc) as tc, ExitStack() as ctx:
        # ---- long-lived pools
        lp = ctx.enter_context(tc.tile_pool(name="longlived", bufs=1))
        ones = lp.tile([128, 1], F32R)
        nc.sync.dma_start(ones[:], ones_in[:].bitcast(F32R))
        eps_q = lp.tile([128, 1], F32)
        nc.gpsimd.memset(eps_q[:], HD * EPS)
        eps_k = lp.tile([128, 1], F32)
        nc.gpsimd.memset(eps_k[:], EPS)
        ident = lp.tile([128, 128], F32)
        make_identity(nc, ident[:])
        cs_t = lp.tile([64, T], F32)           # cos/sin for K side
        nc.sync.dma_start(cs_t[:], cossinT[:])
        cosT, sinT = cs_t[0:32, :], cs_t[32:64, :]
        masks_t = lp.tile([128, 16 * 256], mybir.dt.bfloat16)
        for k in range(16):
            nc.sync.dma_start(masks_t[:, ts(k, 256)], masks[k])
        QT = lp.tile([128, H * QPC], F32R)      # per-head transposed Q [dh, q]
        yT = lp.tile([128, H * QPC], F32R)      # per-head attn out [dh, q]
        sk = lp.tile([128, 32 * HPG], F32)     # K rms scales, [s%128, schunk]
        sk_fm = lp.tile([32, 128 * HPG], F32)  # free-major sumsq [chunk, s%128]
        dram = ctx.enter_context(tc.tile_pool(name="dramp", bufs=2, space="DRAM"))

        gp = ctx.enter_context(tc.tile_pool(name="groups", bufs=1))
        KT = gp.tile([128, HPG * T], F32R, tag="KT")     # [dh, s] per group head
        V = gp.tile([128, (T // 128) * (HPG * HD)], F32R, tag="V")  # 32 x [128, 256]

        # ---------------- phase Q ----------------
        with tc.tile_pool(name="phq", bufs=1) as phq, \
             tc.tile_pool(name="phq_ps", bufs=2, space="PSUM") as qps, \
             tc.tile_pool(name="phq_pst", bufs=2, space="PSUM") as qpst, \
             tc.tile_pool(name="phq_tmp", bufs=3) as qtmp:
            xq = []
            for ic in range(6):
                t = phq.tile([128, QPC], F32R, tag=f"xq{ic}")
                nc.sync.dma_start(t[:], xTq[ts(ic, 128), :].bitcast(F32R))
                xq.append(t)
            wq = []
            for ic in range(6):
                t = phq.tile([128, D], F32R, tag=f"wq{ic}")
                nc.sync.dma_start(t[:], wqT[ts(ic, 128), :].bitcast(F32R))
                wq.append(t)
            csq = []
            for qp in range(4):
                t = phq.tile([128, 64], F32, tag=f"csq{qp}")
                nc.sync.dma_start(t[:], cossinQ[ts(qp, 128), :])
                csq.append(t)

            for qp in range(4 if stage != "setup" else 0):
                for half in range(2):
                    ps = qps.tile([128, 384], F32)
                    for ic in range(6):
                        nc.tensor.matmul(ps[:], (xq[ic][:, ts(qp, 128)]),
                                         (wq[ic][:, ts(half, 384)]),
                                         start=(ic == 0), stop=(ic == 5))
                    for hh in range(3):
                        h = half * 3 + hh
                        qcp = qtmp.tile([128, 128], F32, tag="qcp")
                        nc.vector.tensor_copy(qcp[:], ps[:, ts(hh, 128)])
                        if stage == "qmm":
                            nc.vector.tensor_copy(
                                QT[:, h * QPC + qp * 128: h * QPC + (qp + 1) * 128],
                                qcp[:])
                            continue
                        qsq = qtmp.tile([128, 128], F32, tag="qsq")
                        ssq = qtmp.tile([128, 1], F32, tag="ssq")
                        nc.vector.tensor_mul(qsq[:], qcp[:], qcp[:])
                        nc.vector.tensor_reduce(ssq[:], qsq[:],
                                                mybir.AxisListType.X, ALU.add)
                        sq = qtmp.tile([128, 1], F32, tag="sq")
                        nc.scalar.activation(sq[:], ssq[:], AF.Sqrt, bias=eps_q[:])
                        nc.vector.reciprocal(sq[:], sq[:])
                        qs = qtmp.tile([128, 128], F32, tag="qs")
                        nc.vector.tensor_scalar_mul(qs[:], qcp[:], sq[:])
                        qr = qtmp.tile([128, 128], F32, tag="qr")
                        cosq, sinq = csq[qp][:, 0:32], csq[qp][:, 32:64]
                        t1 = qtmp.tile([128, 32], F32, tag="t1")
                        t2 = qtmp.tile([128, 32], F32, tag="t2")
                        nc.vector.tensor_mul(t1[:], qs[:, 0:32], cosq)
                        nc.vector.tensor_mul(t2[:], qs[:, 64:96], sinq)
                        nc.vector.tensor_add(qr[:, 0:32], t1[:], t2[:])
                        t3 = qtmp.tile([128, 32], F32, tag="t3")
                        t4 = qtmp.tile([128, 32], F32, tag="t4")
                        nc.vector.tensor_mul(t3[:], qs[:, 64:96], cosq)
                        nc.vector.tensor_mul(t4[:], qs[:, 0:32], sinq)
                        nc.vector.tensor_sub(qr[:, 64:96], t3[:], t4[:])
                        nc.vector.tensor_copy(qr[:, 32:64], qs[:, 32:64])
                        nc.vector.tensor_copy(qr[:, 96:128], qs[:, 96:128])
                        if stage == "qnorm":
                            nc.vector.tensor_copy(
                                QT[:, h * QPC + qp * 128: h * QPC + (qp + 1) * 128],
                                qr[:])
                            continue
                        pst = qpst.tile([128, 128], F32)
                        nc.tensor.transpose(pst[:], qr[:], ident[:])
                        nc.vector.tensor_copy(QT[:, h * QPC + qp * 128:
                                                 h * QPC + (qp + 1) * 128], pst[:])

        # ---------------- head-group phases ----------------
        with tc.tile_pool(name="xs", bufs=9) as xsp, \
             tc.tile_pool(name="wg", bufs=1) as wgp, \
             tc.tile_pool(name="vrp", bufs=2) as vrp, \
             tc.tile_pool(name="ktmp", bufs=3) as ktp, \
             tc.tile_pool(name="ksqp", bufs=2) as ksqp, \
             tc.tile_pool(name="atp", bufs=3) as atp, \
             tc.tile_pool(name="smallp", bufs=2) as smp, \
             tc.tile_pool(name="pk", bufs=2, space="PSUM") as pkp, \
             tc.tile_pool(name="pv", bufs=2, space="PSUM") as pvp, \
             tc.tile_pool(name="psc", bufs=2, space="PSUM") as pscp, \
             tc.tile_pool(name="psums", bufs=1, space="PSUM") as psmp, \
             tc.tile_pool(name="py", bufs=1, space="PSUM") as pyp:
            n_groups = (1 if stage in ("kv", "sk", "attn1") else
                        (NG if stage == "full" else 0))
            for g in range(n_groups):
                wk = []
                wv = []
                for ic in range(6):
                    t = wgp.tile([128, HPG * HD], F32R, tag=f"wk{ic}")
                    nc.sync.dma_start(t[:], wkT[ts(ic, 128),
                                                g * HPG * HD:(g + 1) * HPG * HD].bitcast(F32R))
                    wk.append(t)
                    t = wgp.tile([128, HPG * HD], F32R, tag=f"wv{ic}")
                    nc.sync.dma_start(t[:], wvT[ts(ic, 128),
                                                g * HPG * HD:(g + 1) * HPG * HD].bitcast(F32R))
                    wv.append(t)
                # ---- K/V projection, rotary, rms scale
                for sc in range(8):
                    xs = []
                    for ic in range(6):
                        t = xsp.tile([128, 512], F32R, tag="xs")
                        nc.sync.dma_start(t[:], xT[ts(ic, 128), ts(sc, 512)].bitcast(F32R))
                        xs.append(t)
                    for hh in range(HPG):
                        pk = pkp.tile([128, 512], F32)
                        for ic in range(6):
                            nc.tensor.matmul(pk[:], (wk[ic][:, ts(hh, HD)]),
                                             (xs[ic][:]),
                                             start=(ic == 0), stop=(ic == 5))
                        ksl = KT[:, hh * T + sc * 512: hh * T + (sc + 1) * 512]
                        cs, sn = cosT[:, ts(sc, 512)], sinT[:, ts(sc, 512)]
                        t1 = ktp.tile([32, 512], F32, tag="kt1")
                        t2 = ktp.tile([32, 512], F32, tag="kt2")
                        nc.vector.tensor_mul(t1[:], pk[0:32, :], cs)
                        nc.vector.tensor_mul(t2[:], pk[64:96, :], sn)
                        nc.vector.tensor_add(ksl[0:32, :], t1[:], t2[:])
                        t3 = ktp.tile([32, 512], F32, tag="kt1")
                        t4 = ktp.tile([32, 512], F32, tag="kt2")
                        nc.vector.tensor_mul(t3[:], pk[64:96, :], cs)
                        nc.vector.tensor_mul(t4[:], pk[0:32, :], sn)
                        nc.vector.tensor_sub(ksl[64:96, :], t3[:], t4[:])
                        nc.vector.tensor_copy(ksl[32:64, :], pk[32:64, :])
                        nc.vector.tensor_copy(ksl[96:128, :], pk[96:128, :])
                        # rms sumsq over dh (partition dim) via gpsimd
                        ksq = ksqp.tile([128, 512], F32, tag="ksq")
                        nc.gpsimd.tensor_mul(ksq[:], ksl, ksl)
                        strip = ksqp.tile([128, 512], F32, tag="strip")
                        nc.gpsimd.partition_all_reduce(
                            strip[:], ksq[:], channels=128,
                            reduce_op=bass_isa.ReduceOp.add)
                        # bounce [1,512] through DRAM, land as 4 rows of 128
                        dscr = dram.tile([1, 512], F32, tag="dscr")
                        nc.sync.dma_start(dscr[:], strip[0:1, :])
                        nc.sync.dma_start(
                            sk_fm[4 * sc:4 * (sc + 1), hh * 128:(hh + 1) * 128],
                            dscr[0:1, :].rearrange("a (r f) -> (a r) f", r=4))
                    for ss in range(4):
                        pv = pvp.tile([128, HPG * HD], F32)
                        for ic in range(6):
                            nc.tensor.matmul(pv[:], (xs[ic][:, ts(ss, 128)]),
                                             (wv[ic][:]),
                                             start=(ic == 0), stop=(ic == 5))
                        vr = vrp.tile([128, HPG * HD], F32, tag="vr")
                        nc.sync.dma_start(
                            vr[:], vres[sc * 512 + ss * 128: sc * 512 + (ss + 1) * 128,
                                        g * HPG * HD:(g + 1) * HPG * HD])
                        nc.vector.tensor_add(
                            V[:, ts(sc * 4 + ss, HPG * HD)], pv[:], vr[:])
                # finalize rms scales: sk = 1/sqrt(sumsq/128 + eps)
                for hh in range(HPG):
                    pskt = pvp.tile([128, 32], F32, tag="pv")
                    nc.tensor.matmul(pskt[:],
                                     sk_fm[:, hh * 128:(hh + 1) * 128],
                                     ident[0:32, 0:32], is_transpose=True,
                                     start=True, stop=True)
                    nc.scalar.activation(sk[:, hh * 32:(hh + 1) * 32], pskt[:],
                                         AF.Sqrt, scale=1.0 / HD, bias=eps_k[:])
                nc.vector.reciprocal(sk[:], sk[:])
                # ---- attention
                for hh in range(HPG if stage not in ("kv", "sk") else 0):
                    h = g * HPG + hh
                    sums = psmp.tile([1, 512], F32, tag="sums")
                    yac = pyp.tile([128, 512], F32, tag="yac")
                    pend = None
                    for sc in range(32):
                        q0 = 0 if sc < 16 else 256
                        ps = pscp.tile([128, 512], F32)
                        nc.tensor.matmul(ps[:, q0:], (KT[:, hh * T + sc * 128:
                                                           hh * T + (sc + 1) * 128]),
                                         (QT[:, h * QPC + q0: (h + 1) * QPC]),
                                         start=True, stop=True)
                        at = atp.tile([128, 512], F32R, tag="at")
                        nc.scalar.activation(at[:, q0:], ps[:, q0:], AF.Exp,
                                             scale=sk[:, hh * 32 + sc: hh * 32 + sc + 1])
                        k = sc if sc < 16 else sc - 16
                        nc.vector.tensor_mul(at[:, q0:q0 + 256], at[:, q0:q0 + 256],
                                             masks_t[:, ts(k, 256)])
                        if pend is not None:
                            pat, pq0, psc_ = pend
                            nc.tensor.matmul(sums[:, pq0:], (ones[:]), (pat[:, pq0:]),
                                             start=(psc_ == 0), stop=False,
                                             skip_group_check=True)
                            nc.tensor.matmul(yac[:, pq0:],
                                             (V[:, (psc_ * HPG + hh) * HD:
                                                  (psc_ * HPG + hh + 1) * HD]),
                                             (pat[:, pq0:]),
                                             start=(psc_ == 0), stop=False,
                                             skip_group_check=True)
                        pend = (at, q0, sc)
                    pat, pq0, psc_ = pend
                    nc.tensor.matmul(sums[:, pq0:], (ones[:]), (pat[:, pq0:]),
                                     start=False, stop=True, skip_group_check=True)
                    nc.tensor.matmul(yac[:, pq0:],
                                     (V[:, (psc_ * HPG + hh) * HD:
                                          (psc_ * HPG + hh + 1) * HD]),
                                     (pat[:, pq0:]),
                                     start=False, stop=True, skip_group_check=True)
                    rs = smp.tile([1, 512], F32, tag="rs")
                    nc.vector.reciprocal(rs[:], sums[:])
                    rsb = smp.tile([128, 512], F32, tag="rsb")
                    nc.gpsimd.partition_broadcast(rsb[:], rs[:])
                    nc.vector.tensor_mul(yT[:, h * QPC:(h + 1) * QPC], yac[:], rsb[:])

        # ---------------- o_proj ----------------
        if stage != "full":
            dbg = {"q": QT, "kv": KT, "attn1": yT, "setup": cs_t,
                   "qmm": QT, "qnorm": QT, "sk": sk}[stage]
            with tc.tile_pool(name="dbgp", bufs=1) as dbgp:
                np_ = dbg.tensor.shape[0]
                nf_ = min(768, dbg.tensor.shape[-1])
                dt_ = dbgp.tile([128, 768], F32, tag="dt_")
                nc.gpsimd.memset(dt_[:], 0.0)
                nc.vector.tensor_copy(dt_[0:np_, 0:nf_],
                                      dbg[:np_, 0:nf_].bitcast(F32))
                nc.sync.dma_start(out[0:128, :], dt_[:])
        else:
            with tc.tile_pool(name="oph", bufs=1) as oph, \
                 tc.tile_pool(name="op_ps", bufs=2, space="PSUM") as ops, \
                 tc.tile_pool(name="op_out", bufs=3) as opo:
                wo = []
                for ic in range(6):
                    t = oph.tile([128, D], F32R, tag=f"wo{ic}")
                    nc.sync.dma_start(t[:], woT[ts(ic, 128), :].bitcast(F32R))
                    wo.append(t)
                for qp in range(4):
                    for half in range(2):
                        po = ops.tile([128, 384], F32)
                        for hx in range(6):
                            nc.tensor.matmul(po[:],
                                             (yT[:, hx * QPC + qp * 128:
                                                   hx * QPC + (qp + 1) * 128]),
                                             (wo[hx][:, ts(half, 384)]),
                                             start=(hx == 0), stop=(hx == 5))
                        ot = opo.tile([128, 384], F32, tag="ot")
                        nc.vector.tensor_copy(ot[:], po[:])
                        nc.sync.dma_start(out[ts(qp, 128), ts(half, 384)], ot[:])
    nc.compile()
    return nc


def host_prep(x, v_residual, Wq, Wk, Wv, Wo, lamb, pos_id):
    x = np.asarray(x, dtype=np.float32)
    v_residual = np.asarray(v_residual, dtype=np.float32)
    lamb = float(np.asarray(lamb))
    xT = np.ascontiguousarray(x[0].T)
    WqT = np.ascontiguousarray(np.asarray(Wq, dtype=np.float32).T)
    WkT = np.ascontiguousarray(np.asarray(Wk, dtype=np.float32).T)
    WvT = np.ascontiguousarray(np.asarray(Wv, dtype=np.float32).T * (1.0 - lamb))
    WoT = np.ascontiguousarray(np.asarray(Wo, dtype=np.float32).T)
    vres = v_residual[0].reshape(T, H * HD) * lamb
    pos = np.asarray(pos_id).astype(np.float64)
    nf = HD // 4
    af = (1.0 / 1024.0) ** np.linspace(0.0, 1.0, nf)
    theta = np.outer(pos, af)
    cos32 = np.cos(theta).astype(np.float32)
    sin32 = np.sin(theta).astype(np.float32)
    cossinT = np.ascontiguousarray(
        np.concatenate([cos32.T, sin32.T], axis=0))          # [64, T]
    p = np.arange(128)[:, None]
    f = np.arange(256)[None, :]
    in_maps = []
    for c in range(NCORES):
        qidx = c + NCORES * np.arange(QPC)
        in_maps.append({
            "xT": xT,
            "xTq": np.ascontiguousarray(xT[:, qidx]),
            "wqT": WqT, "wkT": WkT, "wvT": WvT, "woT": WoT,
            "vres": np.ascontiguousarray(vres),
            "cossinT": cossinT,
            "cossinQ": np.ascontiguousarray(
                np.concatenate([cos32[qidx], sin32[qidx]], axis=1)),  # [512, 64]
            "ones_in": np.ones((128, 1), np.float32),
            "masks": np.stack([(p - 8 * f <= c - 128 * k) for k in range(16)]
                              ).astype(ml_dtypes.bfloat16),
        })
    return in_maps


_NC_CACHE = {}


def _get_nc():
    if "nc" not in _NC_CACHE:
        _NC_CACHE["nc"] = build_program()
    return _NC_CACHE["nc"]


def run(inputs, trace=False, **kw):
    from concourse.bass_utils import run_bass_kernel_spmd
    nc = _get_nc()
    in_maps = host_prep(inputs["x"], inputs["v_residual"], inputs["Wq"],
                        inputs["Wk"], inputs["Wv"], inputs["Wo"],
                        inputs["lamb"], inputs["pos_id"])
    res = run_bass_kernel_spmd(nc, in_maps, core_ids=list(range(NCORES)),
                               trace=trace, **kw)
    out = np.zeros((1, T, D), dtype=np.float32)
    for c in range(NCORES):
        qidx = c + NCORES * np.arange(QPC)
        out[0, qidx] = res.results[c]["out"]
    return out, res


def kernel(x, v_residual, Wq, Wk, Wv, Wo, lamb, pos_id, mask):
    inputs = dict(x=x, v_residual=v_residual, Wq=Wq, Wk=Wk, Wv=Wv, Wo=Wo,
                  lamb=lamb, pos_id=pos_id, mask=mask)
    out, _ = run(inputs)
    return out, np.asarray(v_residual)
